# revision 31
# baseline (speedup 1.0000x reference)
"""LSTMCell (B=16384, IN=HID=512) on 8 TRN2 NeuronCores.

Strategy: data-parallel over batch (2048 rows/core), weights replicated.
Host pre-packs operands so the device kernel needs zero transposes:
  - GEMM computed as gates.T = W_cat.T @ [x;h].T  (K=1024 on partitions)
  - x/h/W/c cast to bf16 on host (fp32 PSUM accumulation on PE)
  - outputs round to bf16 on-chip (within the 2e-2 max-abs budget)

The PE floor for this GEMM is 512 MMs x 512 cols ~= 110.7us at bf16
(fp8 DoubleRow measured at the same 217ns/instr = only 2x FLOPs, and
the max-abs error budget needs 3 fp8 passes = 1.5x bf16 -> fp8 is out).
The matmul stream must run gap-free at that floor; everything else is
the head (engine prologue ~6.7us + first-chunk DMA) and the tail (last
eviction chain + output DMA + teardown). Design points, all measured
on traces:
  - every DMA region is CONTIGUOUS in DRAM (strided column slices and
    small standalone tensors both measured ~100-120GB/s vs ~250GB/s).
  - one hw-DGE queue sustains ~120-160GB/s (fabric-state dependent),
    and DMA triggers BLOCK the issuing engine's SEQ when the DGE queue
    is full, so the scalar queue (shared with the ACT engine) carries
    only the small early transfers, and the two ACT-table preloads go
    FIRST there (each lazy ACT_TABLE_LOAD is 1.3us; the table loads
    run on the ACT unit while the SEQ issues triggers concurrently).
  - sync queue carries ONLY the sixteen 256KB weight chunks in
    consumption order (splitting them into 128KB halves halved the
    effective queue rate -- per-trigger overhead), then chunks 2/3.
  - phase A: batch-chunk 0 k-outer. First r0+r1 as an r-pair across
    all 8 PSUM banks (8 MMs per weight chunk = 148GB/s demand, the
    queue's capacity; the final k-sweep runs rl-major so r0's banks
    stop ~0.9us early for their eviction chain). Then r2+r3 as a
    STAGGERED second r-pair: r2 alone k=0..4 (its 4 banks free just in
    time), r2+r3 interleaved k=5..7 (halves the rh=1 arrival demand
    exactly where the sync queue is still catching up), r3's k=0..4
    last on long-resident chunks (PSUM accumulation order is free:
    r3 starts its group at k=5 and stops at k=4).
  - ~34 warmup MMs (dep: one memset only) keep the PE activity monitor
    busy from prologue end (~7.1us) so the clock is ramped when the
    first data lands (~10.5-11.5us).
  - per-group gate order (f,i,g,o); the ACT queue is strictly in-order
    and tanh waits on the DVE adds, so o's eviction ACT is emitted
    BEFORE tanh -- banks free ~1.2us earlier for the next group.
    c_new is written bf16 directly by the DVE add (no cast op).
  - the LAST group runs f,g full-width, i and o as half-col bursts
    (second halves in the spare PSUM half's banks) so the i->c->tanh
    chains finish during the o bursts and only ACT-o(256)+mul+trigger
    trail the final MM; its output DMAs queue back-to-back on sync so
    the DGE pipelines their ~1.8us init latencies.
Result: exec ~128-131.5us depending on DMA-fabric state (baseline
129.9-132.5 under the same states); stream gap-free outside the
delivery-bound phase-A window, tail ~5.2us (was 6.3).
"""

import sys

sys.path.insert(0, "/opt/trn_rl_repo")

from contextlib import ExitStack

import ml_dtypes
import numpy as np

import concourse.bass as bass  # noqa: F401  (bass types used via bacc/mybir)
import concourse.mybir as mybir
import concourse.tile as tile
from concourse import bacc
from concourse.bass_utils import run_bass_kernel_spmd

B_FULL, IN, HID = 16384, 512, 512
NCORES = 8
BL = B_FULL // NCORES  # 2048 batch rows per core
JW = 512               # batch columns per chunk (matmul free dim)
P = 128

BF16 = mybir.dt.bfloat16
F32 = mybir.dt.float32
AF = mybir.ActivationFunctionType
BF16_NP = ml_dtypes.bfloat16

NK = (IN + HID) // P   # 8  k-chunks of the contraction dim
NR = HID // P          # 4  row-blocks of H per gate
NM = 4 * HID // P      # 16 gate-row blocks total (i,g,f,o order)

# gate burst order per group: f first (longest elementwise suffix),
# o last (shortest suffix: just ACT + mul + dma)
GATE_ORDER = (2, 0, 1, 3)  # f, i, g, o   (gate index: 0=i 1=g 2=f 3=o)


def build_nc(bl=BL):
    """Build the single-core Bass program (SPMD-replicated across cores)."""
    nbn = bl // JW
    n01 = min(nbn, 2)      # early chunks, k-granular
    n23 = max(nbn - 2, 0)  # late chunks, whole-chunk transfers
    nc = bacc.Bacc("TRN2", target_bir_lowering=False, debug=False)

    # chunk 0 packed k-major: [k][p][jw], each k-slice a contiguous
    # 128KB region (phase A needs per-k completion deps)
    xh0_in = nc.dram_tensor("xh0_in", [NK, P, JW], BF16, kind="ExternalInput")
    xh_in = nc.dram_tensor("xh_in", [max(nbn - 1, 1), P, NK * JW], BF16,
                           kind="ExternalInput")
    # w: contiguous 256KB chunk per (k, rhalf): [k][rh][p][(g*2+rl)*128+j]
    wt_in = nc.dram_tensor("wt_in", [NK, 2, P, 8 * P], BF16, kind="ExternalInput")
    # bias replicated 32x along free dim: 2KB per-partition lines
    bias_in = nc.dram_tensor("bias_in", [P, 32 * NM], F32, kind="ExternalInput")
    # c halves contiguous, bf16: [nb][rh][p][rl*JW + jw]
    c_in = nc.dram_tensor("c_in", [nbn, 2, P, 2 * JW], BF16, kind="ExternalInput")
    # outputs bf16, one contiguous [P, JW] block per (nb, r) group
    h_out = nc.dram_tensor("h_out", [nbn, NR, P, JW], BF16, kind="ExternalOutput")
    c_out = nc.dram_tensor("c_out", [nbn, NR, P, JW], BF16, kind="ExternalOutput")

    with ExitStack() as ctx:
        tc = ctx.enter_context(tile.TileContext(nc))
        wpool = ctx.enter_context(tc.tile_pool(name="w", bufs=1))
        xpool = ctx.enter_context(tc.tile_pool(name="xh", bufs=1))
        cpool = ctx.enter_context(tc.tile_pool(name="cin", bufs=1))
        gpool = ctx.enter_context(tc.tile_pool(name="gates", bufs=3))
        opool = ctx.enter_context(tc.tile_pool(name="outs", bufs=3))
        pspool = ctx.enter_context(tc.tile_pool(name="ps", bufs=1, space="PSUM"))

        ps = [
            pspool.tile([P, JW], F32, tag=f"p{i}", name=f"p{i}") for i in range(8)
        ]

        # --- warmup MMs: clock ramp needs ~3us of continuous PE activity
        wu = wpool.tile([P, P], BF16, tag="wu", name="wu")
        nc.vector.memset(wu[:], 0.0)
        wua = wpool.tile([P, 2], F32, tag="wua", name="wua")
        for _ in range(34):
            nc.tensor.matmul(ps[7][:, :P], wu[:], wu[:], start=True, stop=True)

        # --- SBUF tiles
        x0k = [
            xpool.tile([P, JW], BF16, tag=f"x0k{k}", name=f"x0k{k}")
            for k in range(NK)
        ]
        xh_t = [None] + [
            xpool.tile([P, NK * JW], BF16, tag=f"xh{nb}", name=f"xh{nb}")
            for nb in range(1, nbn)
        ]
        c_t = [
            [
                cpool.tile([P, 2 * JW], BF16, tag=f"c{nb}_{rh}", name=f"c{nb}_{rh}")
                for rh in range(2)
            ]
            for nb in range(nbn)
        ]
        bias_t = wpool.tile([P, 32 * NM], F32, tag="bias", name="bias")

        def xh_sl(nb, k):
            if nb == 0:
                return x0k[k][:]
            return xh_t[nb][:, k * JW : (k + 1) * JW]

        wts = [
            [
                wpool.tile([P, 8 * P], BF16, tag=f"w{k}_{rh}", name=f"w{k}_{rh}")
                for rh in range(2)
            ]
            for k in range(NK)
        ]

        # --- ACT-table preloads FIRST on the scalar queue: they occupy
        # the ACT unit (2x 1.3us) while the SEQ issues DMA triggers.
        nc.scalar.activation(wua[:], wu[:, :2], AF.Sigmoid)
        nc.scalar.activation(wua[:], wu[:, :2], AF.Tanh)
        # sync queue: ONLY weights, in consumption order -- phase A eats
        # one 256KB chunk per 1.73us (148GB/s), the queue's whole
        # measured capacity. Chunks 2/3 + their c follow (first use at
        # S+55us).
        for rh in range(2):
            for k in range(NK):
                nc.sync.dma_start(wts[k][rh][:], wt_in[k, rh])
        for nb in range(2, nbn):
            nc.sync.dma_start(xh_t[nb][:], xh_in[nb - 1])
            nc.sync.dma_start(c_t[nb][0][:], c_in[nb, 0])
            nc.sync.dma_start(c_t[nb][1][:], c_in[nb, 1])
        # scalar queue: x0 k-slices (74GB/s cadence), bias + c0 for the
        # first evictions (~S+12.6 on), then chunk 1 + its c (needed
        # S+27.6 / S+28). Total ~3.25MB, clear by ~33us; at most 3
        # triggers ride the DGE after the eviction-critical point so
        # the SEQ never blocks an eviction ACT on DGE back-pressure.
        for k in range(NK):
            nc.scalar.dma_start(x0k[k][:], xh0_in[k])
        nc.scalar.dma_start(bias_t[:], bias_in[:])
        nc.scalar.dma_start(c_t[0][0][:], c_in[0, 0])
        nc.scalar.dma_start(c_t[0][1][:], c_in[0, 1])
        if nbn > 1:
            nc.scalar.dma_start(xh_t[1][:], xh_in[0])
            nc.scalar.dma_start(c_t[1][0][:], c_in[1, 0])
            nc.scalar.dma_start(c_t[1][1][:], c_in[1, 1])

        def mm_burst(nb, r, gg, base):
            """8 accumulating matmuls (k-inner) for gate gg of (nb, r)."""
            rh, rl = r // 2, r % 2
            col = (gg * 2 + rl) * P
            for k in range(NK):
                nc.tensor.matmul(
                    ps[base + gg][:],
                    wts[k][rh][:, col : col + P],
                    xh_sl(nb, k),
                    start=(k == 0),
                    stop=(k == NK - 1),
                )

        def mm_rpair_kouter(nb, rh):
            """Phase A: k-outer over r-pair (2rh, 2rh+1) across all 8
            PSUM banks, consuming weight chunks in arrival order. Gate g
            of r-local rl -> bank rl*4+g. The final k-sweep runs rl-major
            so r0's four banks all stop ~0.9us earlier -- their eviction
            chain gates the r2 phase's first sweep."""
            for k in range(NK):
                order = (
                    [(gg, rl) for gg in GATE_ORDER for rl in range(2)]
                    if k < NK - 1
                    else [(gg, rl) for rl in range(2) for gg in GATE_ORDER]
                )
                for gg, rl in order:
                    col = (gg * 2 + rl) * P
                    nc.tensor.matmul(
                        ps[rl * 4 + gg][:],
                        wts[k][rh][:, col : col + P],
                        xh_sl(nb, k),
                        start=(k == 0),
                        stop=(k == NK - 1),
                    )

        def mm_group_kouter(nb, r, base):
            """k-outer group: consumes weight chunks in arrival order."""
            rh, rl = r // 2, r % 2
            for k in range(NK):
                for gg in GATE_ORDER:
                    col = (gg * 2 + rl) * P
                    nc.tensor.matmul(
                        ps[base + gg][:],
                        wts[k][rh][:, col : col + P],
                        xh_sl(nb, k),
                        start=(k == 0),
                        stop=(k == NK - 1),
                    )

        def act_gate(nb, r, gg, base, sl, dst):
            fn = AF.Tanh if gg == 1 else AF.Sigmoid
            nc.scalar.activation(
                dst[:, sl], ps[base + gg][:, sl], fn,
                bias=bias_t[:, gg * NR + r : gg * NR + r + 1],
            )

        def elementwise(nb, r, base):
            """Cell update for group (nb, r); gates in ps[base..base+3].
            ACT eviction order f,i,g,o matches the bank-demand order of
            the next group using these banks. The DVE add writes c_new
            as bf16 directly (DMA'd out as-is; tanh reads the bf16)."""
            sl = slice(0, JW)
            cti = c_t[nb][r // 2]
            csl = slice((r % 2) * JW, (r % 2 + 1) * JW)
            ft = gpool.tile([P, JW], F32, tag="f")
            it = gpool.tile([P, JW], F32, tag="i")
            gt = gpool.tile([P, JW], F32, tag="g")
            ot = gpool.tile([P, JW], F32, tag="o")
            t1 = gpool.tile([P, JW], F32, tag="t1")
            t2 = gpool.tile([P, JW], F32, tag="t2")
            tch = gpool.tile([P, JW], F32, tag="tch")
            cnb = opool.tile([P, JW], BF16, tag="cnb")
            hnb = opool.tile([P, JW], BF16, tag="hnb")
            act_gate(nb, r, 2, base, sl, ft)
            nc.vector.tensor_mul(t2[:, sl], ft[:, sl], cti[:, csl])
            act_gate(nb, r, 0, base, sl, it)
            act_gate(nb, r, 1, base, sl, gt)
            nc.vector.tensor_mul(t1[:, sl], it[:, sl], gt[:, sl])
            nc.vector.tensor_add(cnb[:, sl], t1[:, sl], t2[:, sl])
            # o's ACT goes BEFORE tanh: the ACT queue is strictly
            # in-order and tanh waits on the DVE adds, which would hold
            # o's PSUM bank ~1.2us longer than needed.
            act_gate(nb, r, 3, base, sl, ot)
            nc.scalar.activation(tch[:, sl], cnb[:, sl], AF.Tanh)
            nc.sync.dma_start(c_out[nb, r], cnb[:, sl])
            nc.vector.tensor_mul(hnb[:, sl], ot[:, sl], tch[:, sl])
            nc.sync.dma_start(h_out[nb, r], hnb[:, sl])

        def last_group(nb, r, base):
            """Final group: minimize the post-last-MM serial chain."""
            rh, rl = r // 2, r % 2
            ob = 4 - base  # other PSUM half's banks, free by now
            for gg in (2, 1):  # f, g full-width
                mm_burst(nb, r, gg, base)
            cti = c_t[nb][rh]
            ft = gpool.tile([P, JW], F32, tag="f")
            it = gpool.tile([P, JW], F32, tag="i")
            gt = gpool.tile([P, JW], F32, tag="g")
            t1 = gpool.tile([P, JW], F32, tag="t1")
            t2 = gpool.tile([P, JW], F32, tag="t2")
            tch = gpool.tile([P, JW], F32, tag="tch")
            cnb = opool.tile([P, JW], BF16, tag="cnb")
            hnb = opool.tile([P, JW], BF16, tag="hnb")
            ot = gpool.tile([P, JW], F32, tag="o")
            hw_ = JW // 2
            sls = [slice(s * hw_, (s + 1) * hw_) for s in range(2)]
            csls = [slice(rl * JW + s * hw_, rl * JW + (s + 1) * hw_)
                    for s in range(2)]
            colI = (0 * 2 + rl) * P
            colO = (3 * 2 + rl) * P
            ibanks = [ps[base + 0], ps[ob + 0]]
            for s in range(2):
                for k in range(NK):
                    nc.tensor.matmul(
                        ibanks[s][:, sls[s]],
                        wts[k][rh][:, colI : colI + P],
                        xh_sl(nb, k)[:, sls[s]],
                        start=(k == 0),
                        stop=(k == NK - 1),
                    )
            obanks = [ps[base + 3], ps[ob + 3]]
            for s in range(2):
                for k in range(NK):
                    nc.tensor.matmul(
                        obanks[s][:, sls[s]],
                        wts[k][rh][:, colO : colO + P],
                        xh_sl(nb, k)[:, sls[s]],
                        start=(k == 0),
                        stop=(k == NK - 1),
                    )
            for s in range(2):
                act_gate(nb, r, 2, base, sls[s], ft)
                nc.vector.tensor_mul(t2[:, sls[s]], ft[:, sls[s]], cti[:, csls[s]])
            for s in range(2):
                act_gate(nb, r, 1, base, sls[s], gt)
            for s in range(2):
                nc.scalar.activation(
                    it[:, sls[s]], ibanks[s][:, sls[s]], AF.Sigmoid,
                    bias=bias_t[:, 0 * NR + r : 0 * NR + r + 1],
                )
                nc.vector.tensor_mul(t1[:, sls[s]], it[:, sls[s]], gt[:, sls[s]])
                nc.vector.tensor_add(cnb[:, sls[s]], t1[:, sls[s]], t2[:, sls[s]])
                nc.sync.dma_start(c_out[nb, r][:, sls[s]], cnb[:, sls[s]])
            for s in range(2):
                nc.scalar.activation(tch[:, sls[s]], cnb[:, sls[s]], AF.Tanh)
                nc.scalar.activation(
                    ot[:, sls[s]], obanks[s][:, sls[s]], AF.Sigmoid,
                    bias=bias_t[:, 3 * NR + r : 3 * NR + r + 1],
                )
                nc.vector.tensor_mul(hnb[:, sls[s]], ot[:, sls[s]], tch[:, sls[s]])
                nc.sync.dma_start(h_out[nb, r][:, sls[s]], hnb[:, sls[s]])

        # --- phase A: batch-chunk 0 entirely k-outer, consuming weight
        # chunks in arrival order. First the r0+r1 pair across all 8
        # banks, then r2 (r0's banks, freed gate-by-gate in matching
        # order) and r3 (r1's banks, freed long before).
        def phase_a2():
            """r2+r3 staggered second r-pair: r2 k-outer alone for
            k=0..4 (r0's banks free just in time; chunks banked), then
            k=5..7 interleaved with r3 (halves the rh=1 arrival demand
            right where the sync queue is still catching up), then r3's
            k=0..4 on long-resident chunks. Accumulation order within a
            bank is arbitrary: r3 starts its group at k=5 and stops at
            k=4."""
            def sweep(r, base, k, start, stop):
                rl = r % 2
                for gg in GATE_ORDER:
                    col = (gg * 2 + rl) * P
                    nc.tensor.matmul(
                        ps[base + gg][:],
                        wts[k][1][:, col : col + P],
                        xh_sl(0, k),
                        start=start,
                        stop=stop,
                    )
            for k in range(5):
                sweep(2, 0, k, k == 0, False)
            for k in range(5, NK):
                sweep(2, 0, k, False, k == NK - 1)
                sweep(3, 4, k, k == 5, False)
            for k in range(5):
                sweep(3, 4, k, False, k == 4)

        mm_rpair_kouter(0, 0)
        elementwise(0, 0, base=0)
        if nbn == 1:
            mm_group_kouter(0, 2, base=0)
            elementwise(0, 1, base=4)
            elementwise(0, 2, base=0)
            last_group(0, 3, base=4)
        else:
            elementwise(0, 1, base=4)
            phase_a2()
            elementwise(0, 2, base=0)
            elementwise(0, 3, base=4)

            # --- steady state: remaining groups, g-outer k-inner,
            # alternating PSUM halves for double buffering.
            groups = [
                (nb, r) for nb in range(1, nbn) for r in range(NR)
            ]
            for j, (nb, r) in enumerate(groups):
                base = 4 * (j % 2)
                if j == len(groups) - 1:
                    last_group(nb, r, base)
                else:
                    for gg in GATE_ORDER:
                        mm_burst(nb, r, gg, base)
                    elementwise(nb, r, base)

    nc.compile()
    return nc


def prep_shared(Wxi, Wxg, Wxf, Wxo, Whi, Whg, Whf, Who, bias_sum):
    """wt_in [NK,2,P,8P] bf16 and bias_in [P,32*NM] f32 (gate order i,g,f,o)."""
    Wx = np.concatenate([Wxi, Wxg, Wxf, Wxo], axis=0)  # [4H, IN]
    Wh = np.concatenate([Whi, Whg, Whf, Who], axis=0)  # [4H, HID]
    WT = np.concatenate([Wx.T, Wh.T], axis=0)          # [K=1024, 4H]
    W6 = WT.reshape(NK, P, 4, 2, 2, P)        # [k, p, g, rh, rl, j]
    wt_arr = np.ascontiguousarray(
        W6.transpose(0, 3, 1, 2, 4, 5)        # [k, rh, p, g, rl, j]
        .reshape(NK, 2, P, 8 * P)
        .astype(BF16_NP)
    )
    bias_arr = np.ascontiguousarray(
        np.tile(bias_sum.reshape(NM, P).T.astype(np.float32), (1, 32))
    )
    return wt_arr, bias_arr


def prep_core(x_s, h_s, c_s):
    """Per-core xh [nbn,P,NK*JW] bf16 and c [nbn,2,P,2*JW] bf16."""
    bl = x_s.shape[0]
    nbn = bl // JW
    xhT = np.concatenate([x_s, h_s], axis=1).T  # [K=1024, bl]
    xh_arr = np.ascontiguousarray(
        xhT.reshape(NK, P, nbn, JW).transpose(2, 1, 0, 3)
        .reshape(nbn, P, NK * JW)
        .astype(BF16_NP)
    )
    cT = c_s.T  # [HID, bl]
    c_arr = np.ascontiguousarray(
        cT.reshape(2, 2, P, nbn, JW).transpose(3, 0, 2, 1, 4)
        .reshape(nbn, 2, P, 2 * JW)
        .astype(BF16_NP)
    )
    return xh_arr, c_arr


def split_xh(xh_arr):
    """xh [nbn,P,NK*JW] -> (xh0 [NK,P,JW], xh_rest [nbn-1,P,NK*JW])."""
    nbn = xh_arr.shape[0]
    xh0 = np.ascontiguousarray(
        xh_arr[0].reshape(P, NK, JW).transpose(1, 0, 2)
    )
    if nbn > 1:
        rest = np.ascontiguousarray(xh_arr[1:])
    else:
        rest = np.zeros((1, P, NK * JW), BF16_NP)
    return xh0, rest


def post_core(arr):
    """[nbn,NR,P,JW] -> [bl, HID]"""
    arr = np.asarray(arr)
    nbn = arr.size // (NR * P * JW)
    arr = arr.reshape(nbn, NR, P, JW)
    return arr.transpose(0, 3, 1, 2).reshape(nbn * JW, HID)


_NC_CACHE = {}


def _get_nc(bl=BL):
    if bl not in _NC_CACHE:
        _NC_CACHE[bl] = build_nc(bl)
    return _NC_CACHE[bl]


def make_in_maps(x, h, c, Wxi, bxi, Wxo, bxo, Wxf, bxf, Wxg, bxg,
                 Whi, bhi, Who, bho, Whf, bhf, Whg, bhg, ncores=NCORES):
    bias_sum = np.concatenate(
        [bxi + bhi, bxg + bhg, bxf + bhf, bxo + bho], axis=0
    ).astype(np.float32)
    wt_arr, bias_arr = prep_shared(Wxi, Wxg, Wxf, Wxo, Whi, Whg, Whf, Who, bias_sum)
    bl = x.shape[0] // ncores
    in_maps = []
    for i in range(ncores):
        s = slice(i * bl, (i + 1) * bl)
        xh_arr, c_arr = prep_core(
            np.asarray(x[s], np.float32),
            np.asarray(h[s], np.float32),
            np.asarray(c[s], np.float32),
        )
        xh0, rest = split_xh(xh_arr)
        in_maps.append(
            {
                "xh0_in": xh0,
                "xh_in": rest,
                "wt_in": wt_arr,
                "bias_in": bias_arr,
                "c_in": c_arr,
            }
        )
    return in_maps


def kernel(x, h, c, Wxi, bxi, Wxo, bxo, Wxf, bxf, Wxg, bxg,
           Whi, bhi, Who, bho, Whf, bhf, Whg, bhg):
    args = dict(
        x=np.asarray(x, np.float32), h=np.asarray(h, np.float32),
        c=np.asarray(c, np.float32),
        Wxi=np.asarray(Wxi, np.float32), bxi=np.asarray(bxi, np.float32),
        Wxo=np.asarray(Wxo, np.float32), bxo=np.asarray(bxo, np.float32),
        Wxf=np.asarray(Wxf, np.float32), bxf=np.asarray(bxf, np.float32),
        Wxg=np.asarray(Wxg, np.float32), bxg=np.asarray(bxg, np.float32),
        Whi=np.asarray(Whi, np.float32), bhi=np.asarray(bhi, np.float32),
        Who=np.asarray(Who, np.float32), bho=np.asarray(bho, np.float32),
        Whf=np.asarray(Whf, np.float32), bhf=np.asarray(bhf, np.float32),
        Whg=np.asarray(Whg, np.float32), bhg=np.asarray(bhg, np.float32),
    )
    in_maps = make_in_maps(**args)
    nc = _get_nc(BL)
    res = run_bass_kernel_spmd(nc, in_maps, core_ids=list(range(NCORES)))
    h_new = np.empty((B_FULL, HID), np.float32)
    c_new = np.empty((B_FULL, HID), np.float32)
    for i in range(NCORES):
        s = slice(i * BL, (i + 1) * BL)
        h_new[s] = post_core(res.results[i]["h_out"])
        c_new[s] = post_core(res.results[i]["c_out"])
    return (h_new, c_new)


# revision 32
# speedup vs baseline: 1.1626x; 1.1626x over previous
"""LSTMCell (B=16384, IN=HID=512) on 8 TRN2 NeuronCores.

Strategy: data-parallel over batch (2048 rows/core), weights replicated.
Host pre-packs operands so the device kernel needs zero transposes:
  - GEMM computed as gates.T = W_cat.T @ [x;h].T  (K=1024 on partitions)
  - x/h/W/c cast to bf16 on host (fp32 PSUM accumulation on PE)
  - outputs round to bf16 on-chip (within the 2e-2 max-abs budget)

The PE floor for this GEMM is 512 MMs x 512 cols ~= 110.7us at bf16
(fp8 DoubleRow measured at the same 217ns/instr = only 2x FLOPs, and
the max-abs error budget needs 3 fp8 passes = 1.5x bf16 -> fp8 is out).
The matmul stream must run gap-free at that floor; everything else is
the head (engine prologue ~6.7us + first-chunk DMA) and the tail (last
eviction chain + output DMA + teardown). Design points, all measured
on traces:
  - every DMA region is CONTIGUOUS in DRAM (strided column slices and
    small standalone tensors both measured ~100-120GB/s vs ~250GB/s).
  - one hw-DGE queue sustains ~120-160GB/s (fabric-state dependent),
    and DMA triggers BLOCK the issuing engine's SEQ when the DGE queue
    is full, so the scalar queue (shared with the ACT engine) carries
    only the small early transfers, and the two ACT-table preloads go
    FIRST there (each lazy ACT_TABLE_LOAD is 1.3us; the table loads
    run on the ACT unit while the SEQ issues triggers concurrently).
  - sync queue carries ONLY the sixteen 256KB weight chunks in
    consumption order (splitting them into 128KB halves halved the
    effective queue rate -- per-trigger overhead), then chunks 2/3.
  - phase A: batch-chunk 0 k-outer. First r0+r1 as an r-pair across
    all 8 PSUM banks (8 MMs per weight chunk = 148GB/s demand, the
    queue's capacity; the final k-sweep runs rl-major so r0's banks
    stop ~0.9us early for their eviction chain). Then r2+r3 as a
    STAGGERED second r-pair: r2 alone k=0..4 (its 4 banks free just in
    time), r2+r3 interleaved k=5..7 (halves the rh=1 arrival demand
    exactly where the sync queue is still catching up), r3's k=0..4
    last on long-resident chunks (PSUM accumulation order is free:
    r3 starts its group at k=5 and stops at k=4).
  - ~34 warmup MMs (dep: one memset only) keep the PE activity monitor
    busy from prologue end (~7.1us) so the clock is ramped when the
    first data lands (~10.5-11.5us).
  - per-group gate order (f,i,g,o); the ACT queue is strictly in-order
    and tanh waits on the DVE adds, so o's eviction ACT is emitted
    BEFORE tanh -- banks free ~1.2us earlier for the next group.
    c_new is written bf16 directly by the DVE add (no cast op).
  - the LAST group runs f,g full-width, i and o as half-col bursts
    (second halves in the spare PSUM half's banks) so the i->c->tanh
    chains finish during the o bursts and only ACT-o(256)+mul+trigger
    trail the final MM; its output DMAs queue back-to-back on sync so
    the DGE pipelines their ~1.8us init latencies.
Result: exec ~128-131.5us depending on DMA-fabric state (baseline
129.9-132.5 under the same states); stream gap-free outside the
delivery-bound phase-A window, tail ~5.2us (was 6.3).
"""

import sys

sys.path.insert(0, "/opt/trn_rl_repo")

from contextlib import ExitStack

import ml_dtypes
import numpy as np

import concourse.bass as bass  # noqa: F401  (bass types used via bacc/mybir)
import concourse.mybir as mybir
import concourse.tile as tile
from concourse import bacc
from concourse.bass_utils import run_bass_kernel_spmd

B_FULL, IN, HID = 16384, 512, 512
NCORES = 8
BL = B_FULL // NCORES  # 2048 batch rows per core
JW = 512               # batch columns per chunk (matmul free dim)
P = 128

BF16 = mybir.dt.bfloat16
F32 = mybir.dt.float32
AF = mybir.ActivationFunctionType
BF16_NP = ml_dtypes.bfloat16

NK = (IN + HID) // P   # 8  k-chunks of the contraction dim
NR = HID // P          # 4  row-blocks of H per gate
NM = 4 * HID // P      # 16 gate-row blocks total (i,g,f,o order)

# gate burst order per group: f first (longest elementwise suffix),
# o last (shortest suffix: just ACT + mul + dma)
GATE_ORDER = (2, 0, 1, 3)  # f, i, g, o   (gate index: 0=i 1=g 2=f 3=o)


def build_nc(bl=BL):
    """Build the single-core Bass program (SPMD-replicated across cores)."""
    nbn = bl // JW
    n01 = min(nbn, 2)      # early chunks, k-granular
    n23 = max(nbn - 2, 0)  # late chunks, whole-chunk transfers
    nc = bacc.Bacc("TRN2", target_bir_lowering=False, debug=False)

    # chunk 0 packed k-major: [k][p][jw], each k-slice a contiguous
    # 128KB region (phase A needs per-k completion deps)
    xh0_in = nc.dram_tensor("xh0_in", [NK, P, JW], BF16, kind="ExternalInput")
    xh_in = nc.dram_tensor("xh_in", [max(nbn - 1, 1), P, NK * JW], BF16,
                           kind="ExternalInput")
    # w: contiguous 256KB chunk per (k, rhalf): [k][rh][p][(g*2+rl)*128+j]
    wt_in = nc.dram_tensor("wt_in", [NK, 2, P, 8 * P], BF16, kind="ExternalInput")
    # bias replicated 32x along free dim: 2KB per-partition lines
    bias_in = nc.dram_tensor("bias_in", [P, 32 * NM], F32, kind="ExternalInput")
    # c halves contiguous, bf16: [nb][rh][p][rl*JW + jw]
    c_in = nc.dram_tensor("c_in", [nbn, 2, P, 2 * JW], BF16, kind="ExternalInput")
    # outputs bf16, one contiguous [P, JW] block per (nb, r) group
    h_out = nc.dram_tensor("h_out", [nbn, NR, P, JW], BF16, kind="ExternalOutput")
    c_out = nc.dram_tensor("c_out", [nbn, NR, P, JW], BF16, kind="ExternalOutput")

    with ExitStack() as ctx:
        tc = ctx.enter_context(tile.TileContext(nc))
        wpool = ctx.enter_context(tc.tile_pool(name="w", bufs=1))
        xpool = ctx.enter_context(tc.tile_pool(name="xh", bufs=1))
        cpool = ctx.enter_context(tc.tile_pool(name="cin", bufs=1))
        gpool = ctx.enter_context(tc.tile_pool(name="gates", bufs=3))
        opool = ctx.enter_context(tc.tile_pool(name="outs", bufs=3))
        pspool = ctx.enter_context(tc.tile_pool(name="ps", bufs=1, space="PSUM"))

        ps = [
            pspool.tile([P, JW], F32, tag=f"p{i}", name=f"p{i}") for i in range(8)
        ]

        # --- warmup MMs: clock ramp needs ~3us of continuous PE activity
        wu = wpool.tile([P, P], BF16, tag="wu", name="wu")
        nc.vector.memset(wu[:], 0.0)
        wua = wpool.tile([P, 2], F32, tag="wua", name="wua")
        for _ in range(34):
            nc.tensor.matmul(ps[7][:, :P], wu[:], wu[:], start=True, stop=True)

        # --- SBUF tiles
        x0k = [
            xpool.tile([P, JW], BF16, tag=f"x0k{k}", name=f"x0k{k}")
            for k in range(NK)
        ]
        xh_t = [None] + [
            xpool.tile([P, NK * JW], BF16, tag=f"xh{nb}", name=f"xh{nb}")
            for nb in range(1, nbn)
        ]
        c_t = [
            [
                cpool.tile([P, 2 * JW], BF16, tag=f"c{nb}_{rh}", name=f"c{nb}_{rh}")
                for rh in range(2)
            ]
            for nb in range(nbn)
        ]
        bias_t = wpool.tile([P, 32 * NM], F32, tag="bias", name="bias")

        def xh_sl(nb, k):
            if nb == 0:
                return x0k[k][:]
            return xh_t[nb][:, k * JW : (k + 1) * JW]

        wts = [
            [
                wpool.tile([P, 8 * P], BF16, tag=f"w{k}_{rh}", name=f"w{k}_{rh}")
                for rh in range(2)
            ]
            for k in range(NK)
        ]

        # --- ACT-table preloads FIRST on the scalar queue: they occupy
        # the ACT unit (2x 1.3us) while the SEQ issues DMA triggers.
        nc.scalar.activation(wua[:], wu[:, :2], AF.Sigmoid)
        nc.scalar.activation(wua[:], wu[:, :2], AF.Tanh)
        # sync queue: ONLY weights, in consumption order -- phase A eats
        # one 256KB chunk per 1.73us (148GB/s), the queue's whole
        # measured capacity. Chunks 2/3 + their c follow (first use at
        # S+55us).
        for k in range(NK):
            nc.sync.dma_start(wts[k][0][:], wt_in[k, 0])
        # first two rh=1 chunks ride the scalar queue (it drains its
        # early load ~3us before r2 needs them; sync is still ~2MB
        # deep in rh=0 at that point) -- rest of rh=1 on sync.
        for k in range(2, NK):
            nc.sync.dma_start(wts[k][1][:], wt_in[k, 1])
        for nb in range(2, nbn):
            nc.sync.dma_start(xh_t[nb][:], xh_in[nb - 1])
            nc.sync.dma_start(c_t[nb][0][:], c_in[nb, 0])
            nc.sync.dma_start(c_t[nb][1][:], c_in[nb, 1])
        # scalar queue: x0 k-slices (74GB/s cadence), bias + c0 for the
        # first evictions (~S+12.6 on), then chunk 1 + its c (needed
        # S+27.6 / S+28). Total ~3.25MB, clear by ~33us; at most 3
        # triggers ride the DGE after the eviction-critical point so
        # the SEQ never blocks an eviction ACT on DGE back-pressure.
        for k in range(NK):
            nc.scalar.dma_start(x0k[k][:], xh0_in[k])
        nc.scalar.dma_start(bias_t[:], bias_in[:])
        nc.scalar.dma_start(c_t[0][0][:], c_in[0, 0])
        nc.scalar.dma_start(c_t[0][1][:], c_in[0, 1])
        nc.scalar.dma_start(wts[0][1][:], wt_in[0, 1])
        nc.scalar.dma_start(wts[1][1][:], wt_in[1, 1])
        if nbn > 1:
            nc.scalar.dma_start(xh_t[1][:], xh_in[0])
            nc.scalar.dma_start(c_t[1][0][:], c_in[1, 0])
            nc.scalar.dma_start(c_t[1][1][:], c_in[1, 1])

        def mm_burst(nb, r, gg, base):
            """8 accumulating matmuls (k-inner) for gate gg of (nb, r)."""
            rh, rl = r // 2, r % 2
            col = (gg * 2 + rl) * P
            for k in range(NK):
                nc.tensor.matmul(
                    ps[base + gg][:],
                    wts[k][rh][:, col : col + P],
                    xh_sl(nb, k),
                    start=(k == 0),
                    stop=(k == NK - 1),
                )

        def mm_rpair_kouter(nb, rh):
            """Phase A: k-outer over r-pair (2rh, 2rh+1) across all 8
            PSUM banks, consuming weight chunks in arrival order. Gate g
            of r-local rl -> bank rl*4+g. The final k-sweep runs rl-major
            so r0's four banks all stop ~0.9us earlier -- their eviction
            chain gates the r2 phase's first sweep."""
            for k in range(NK):
                order = (
                    [(gg, rl) for gg in GATE_ORDER for rl in range(2)]
                    if k < NK - 1
                    else [(gg, rl) for rl in range(2) for gg in GATE_ORDER]
                )
                for gg, rl in order:
                    col = (gg * 2 + rl) * P
                    nc.tensor.matmul(
                        ps[rl * 4 + gg][:],
                        wts[k][rh][:, col : col + P],
                        xh_sl(nb, k),
                        start=(k == 0),
                        stop=(k == NK - 1),
                    )

        def mm_group_kouter(nb, r, base):
            """k-outer group: consumes weight chunks in arrival order."""
            rh, rl = r // 2, r % 2
            for k in range(NK):
                for gg in GATE_ORDER:
                    col = (gg * 2 + rl) * P
                    nc.tensor.matmul(
                        ps[base + gg][:],
                        wts[k][rh][:, col : col + P],
                        xh_sl(nb, k),
                        start=(k == 0),
                        stop=(k == NK - 1),
                    )

        def act_gate(nb, r, gg, base, sl, dst):
            fn = AF.Tanh if gg == 1 else AF.Sigmoid
            nc.scalar.activation(
                dst[:, sl], ps[base + gg][:, sl], fn,
                bias=bias_t[:, gg * NR + r : gg * NR + r + 1],
            )

        def elementwise(nb, r, base):
            """Cell update for group (nb, r); gates in ps[base..base+3].
            ACT eviction order f,i,g,o matches the bank-demand order of
            the next group using these banks. The DVE add writes c_new
            as bf16 directly (DMA'd out as-is; tanh reads the bf16)."""
            sl = slice(0, JW)
            cti = c_t[nb][r // 2]
            csl = slice((r % 2) * JW, (r % 2 + 1) * JW)
            ft = gpool.tile([P, JW], F32, tag="f")
            it = gpool.tile([P, JW], F32, tag="i")
            gt = gpool.tile([P, JW], F32, tag="g")
            ot = gpool.tile([P, JW], F32, tag="o")
            t1 = gpool.tile([P, JW], F32, tag="t1")
            t2 = gpool.tile([P, JW], F32, tag="t2")
            tch = gpool.tile([P, JW], F32, tag="tch")
            cnb = opool.tile([P, JW], BF16, tag="cnb")
            hnb = opool.tile([P, JW], BF16, tag="hnb")
            act_gate(nb, r, 2, base, sl, ft)
            nc.vector.tensor_mul(t2[:, sl], ft[:, sl], cti[:, csl])
            act_gate(nb, r, 0, base, sl, it)
            act_gate(nb, r, 1, base, sl, gt)
            nc.vector.tensor_mul(t1[:, sl], it[:, sl], gt[:, sl])
            nc.vector.tensor_add(cnb[:, sl], t1[:, sl], t2[:, sl])
            # o's ACT goes BEFORE tanh: the ACT queue is strictly
            # in-order and tanh waits on the DVE adds, which would hold
            # o's PSUM bank ~1.2us longer than needed.
            act_gate(nb, r, 3, base, sl, ot)
            nc.scalar.activation(tch[:, sl], cnb[:, sl], AF.Tanh)
            nc.sync.dma_start(c_out[nb, r], cnb[:, sl])
            nc.vector.tensor_mul(hnb[:, sl], ot[:, sl], tch[:, sl])
            nc.sync.dma_start(h_out[nb, r], hnb[:, sl])

        def last_group(nb, r, base):
            """Final group: minimize the post-last-MM serial chain."""
            rh, rl = r // 2, r % 2
            ob = 4 - base  # other PSUM half's banks, free by now
            for gg in (2, 1):  # f, g full-width
                mm_burst(nb, r, gg, base)
            cti = c_t[nb][rh]
            ft = gpool.tile([P, JW], F32, tag="f")
            it = gpool.tile([P, JW], F32, tag="i")
            gt = gpool.tile([P, JW], F32, tag="g")
            t1 = gpool.tile([P, JW], F32, tag="t1")
            t2 = gpool.tile([P, JW], F32, tag="t2")
            tch = gpool.tile([P, JW], F32, tag="tch")
            cnb = opool.tile([P, JW], BF16, tag="cnb")
            hnb = opool.tile([P, JW], BF16, tag="hnb")
            ot = gpool.tile([P, JW], F32, tag="o")
            hw_ = JW // 2
            sls = [slice(s * hw_, (s + 1) * hw_) for s in range(2)]
            csls = [slice(rl * JW + s * hw_, rl * JW + (s + 1) * hw_)
                    for s in range(2)]
            colI = (0 * 2 + rl) * P
            colO = (3 * 2 + rl) * P
            ibanks = [ps[base + 0], ps[ob + 0]]
            for s in range(2):
                for k in range(NK):
                    nc.tensor.matmul(
                        ibanks[s][:, sls[s]],
                        wts[k][rh][:, colI : colI + P],
                        xh_sl(nb, k)[:, sls[s]],
                        start=(k == 0),
                        stop=(k == NK - 1),
                    )
            obanks = [ps[base + 3], ps[ob + 3]]
            for s in range(2):
                for k in range(NK):
                    nc.tensor.matmul(
                        obanks[s][:, sls[s]],
                        wts[k][rh][:, colO : colO + P],
                        xh_sl(nb, k)[:, sls[s]],
                        start=(k == 0),
                        stop=(k == NK - 1),
                    )
            for s in range(2):
                act_gate(nb, r, 2, base, sls[s], ft)
                nc.vector.tensor_mul(t2[:, sls[s]], ft[:, sls[s]], cti[:, csls[s]])
            for s in range(2):
                act_gate(nb, r, 1, base, sls[s], gt)
            for s in range(2):
                nc.scalar.activation(
                    it[:, sls[s]], ibanks[s][:, sls[s]], AF.Sigmoid,
                    bias=bias_t[:, 0 * NR + r : 0 * NR + r + 1],
                )
                nc.vector.tensor_mul(t1[:, sls[s]], it[:, sls[s]], gt[:, sls[s]])
                nc.vector.tensor_add(cnb[:, sls[s]], t1[:, sls[s]], t2[:, sls[s]])
                nc.sync.dma_start(c_out[nb, r][:, sls[s]], cnb[:, sls[s]])
            for s in range(2):
                nc.scalar.activation(tch[:, sls[s]], cnb[:, sls[s]], AF.Tanh)
                nc.scalar.activation(
                    ot[:, sls[s]], obanks[s][:, sls[s]], AF.Sigmoid,
                    bias=bias_t[:, 3 * NR + r : 3 * NR + r + 1],
                )
                nc.vector.tensor_mul(hnb[:, sls[s]], ot[:, sls[s]], tch[:, sls[s]])
                nc.sync.dma_start(h_out[nb, r][:, sls[s]], hnb[:, sls[s]])

        # --- phase A: batch-chunk 0 entirely k-outer, consuming weight
        # chunks in arrival order. First the r0+r1 pair across all 8
        # banks, then r2 (r0's banks, freed gate-by-gate in matching
        # order) and r3 (r1's banks, freed long before).
        def phase_a2():
            """r2+r3 staggered second r-pair: r2 k-outer alone for
            k=0..4 (r0's banks free just in time; chunks banked), then
            k=5..7 interleaved with r3 (halves the rh=1 arrival demand
            right where the sync queue is still catching up), then r3's
            k=0..4 on long-resident chunks. Accumulation order within a
            bank is arbitrary: r3 starts its group at k=5 and stops at
            k=4."""
            def sweep(r, base, k, start, stop):
                rl = r % 2
                for gg in GATE_ORDER:
                    col = (gg * 2 + rl) * P
                    nc.tensor.matmul(
                        ps[base + gg][:],
                        wts[k][1][:, col : col + P],
                        xh_sl(0, k),
                        start=start,
                        stop=stop,
                    )
            for k in range(5):
                sweep(2, 0, k, k == 0, False)
            for k in range(5, NK):
                sweep(2, 0, k, False, k == NK - 1)
                sweep(3, 4, k, k == 5, False)
            for k in range(5):
                sweep(3, 4, k, False, k == 4)

        mm_rpair_kouter(0, 0)
        elementwise(0, 0, base=0)
        if nbn == 1:
            mm_group_kouter(0, 2, base=0)
            elementwise(0, 1, base=4)
            elementwise(0, 2, base=0)
            last_group(0, 3, base=4)
        else:
            elementwise(0, 1, base=4)
            phase_a2()
            elementwise(0, 2, base=0)
            elementwise(0, 3, base=4)

            # --- steady state: remaining groups, g-outer k-inner,
            # alternating PSUM halves for double buffering.
            groups = [
                (nb, r) for nb in range(1, nbn) for r in range(NR)
            ]
            for j, (nb, r) in enumerate(groups):
                base = 4 * (j % 2)
                if j == len(groups) - 1:
                    last_group(nb, r, base)
                else:
                    for gg in GATE_ORDER:
                        mm_burst(nb, r, gg, base)
                    elementwise(nb, r, base)

    nc.compile()
    return nc


def prep_shared(Wxi, Wxg, Wxf, Wxo, Whi, Whg, Whf, Who, bias_sum):
    """wt_in [NK,2,P,8P] bf16 and bias_in [P,32*NM] f32 (gate order i,g,f,o)."""
    Wx = np.concatenate([Wxi, Wxg, Wxf, Wxo], axis=0)  # [4H, IN]
    Wh = np.concatenate([Whi, Whg, Whf, Who], axis=0)  # [4H, HID]
    WT = np.concatenate([Wx.T, Wh.T], axis=0)          # [K=1024, 4H]
    W6 = WT.reshape(NK, P, 4, 2, 2, P)        # [k, p, g, rh, rl, j]
    wt_arr = np.ascontiguousarray(
        W6.transpose(0, 3, 1, 2, 4, 5)        # [k, rh, p, g, rl, j]
        .reshape(NK, 2, P, 8 * P)
        .astype(BF16_NP)
    )
    bias_arr = np.ascontiguousarray(
        np.tile(bias_sum.reshape(NM, P).T.astype(np.float32), (1, 32))
    )
    return wt_arr, bias_arr


def prep_core(x_s, h_s, c_s):
    """Per-core xh [nbn,P,NK*JW] bf16 and c [nbn,2,P,2*JW] bf16."""
    bl = x_s.shape[0]
    nbn = bl // JW
    xhT = np.concatenate([x_s, h_s], axis=1).T  # [K=1024, bl]
    xh_arr = np.ascontiguousarray(
        xhT.reshape(NK, P, nbn, JW).transpose(2, 1, 0, 3)
        .reshape(nbn, P, NK * JW)
        .astype(BF16_NP)
    )
    cT = c_s.T  # [HID, bl]
    c_arr = np.ascontiguousarray(
        cT.reshape(2, 2, P, nbn, JW).transpose(3, 0, 2, 1, 4)
        .reshape(nbn, 2, P, 2 * JW)
        .astype(BF16_NP)
    )
    return xh_arr, c_arr


def split_xh(xh_arr):
    """xh [nbn,P,NK*JW] -> (xh0 [NK,P,JW], xh_rest [nbn-1,P,NK*JW])."""
    nbn = xh_arr.shape[0]
    xh0 = np.ascontiguousarray(
        xh_arr[0].reshape(P, NK, JW).transpose(1, 0, 2)
    )
    if nbn > 1:
        rest = np.ascontiguousarray(xh_arr[1:])
    else:
        rest = np.zeros((1, P, NK * JW), BF16_NP)
    return xh0, rest


def post_core(arr):
    """[nbn,NR,P,JW] -> [bl, HID]"""
    arr = np.asarray(arr)
    nbn = arr.size // (NR * P * JW)
    arr = arr.reshape(nbn, NR, P, JW)
    return arr.transpose(0, 3, 1, 2).reshape(nbn * JW, HID)


_NC_CACHE = {}


def _get_nc(bl=BL):
    if bl not in _NC_CACHE:
        _NC_CACHE[bl] = build_nc(bl)
    return _NC_CACHE[bl]


def make_in_maps(x, h, c, Wxi, bxi, Wxo, bxo, Wxf, bxf, Wxg, bxg,
                 Whi, bhi, Who, bho, Whf, bhf, Whg, bhg, ncores=NCORES):
    bias_sum = np.concatenate(
        [bxi + bhi, bxg + bhg, bxf + bhf, bxo + bho], axis=0
    ).astype(np.float32)
    wt_arr, bias_arr = prep_shared(Wxi, Wxg, Wxf, Wxo, Whi, Whg, Whf, Who, bias_sum)
    bl = x.shape[0] // ncores
    in_maps = []
    for i in range(ncores):
        s = slice(i * bl, (i + 1) * bl)
        xh_arr, c_arr = prep_core(
            np.asarray(x[s], np.float32),
            np.asarray(h[s], np.float32),
            np.asarray(c[s], np.float32),
        )
        xh0, rest = split_xh(xh_arr)
        in_maps.append(
            {
                "xh0_in": xh0,
                "xh_in": rest,
                "wt_in": wt_arr,
                "bias_in": bias_arr,
                "c_in": c_arr,
            }
        )
    return in_maps


def kernel(x, h, c, Wxi, bxi, Wxo, bxo, Wxf, bxf, Wxg, bxg,
           Whi, bhi, Who, bho, Whf, bhf, Whg, bhg):
    args = dict(
        x=np.asarray(x, np.float32), h=np.asarray(h, np.float32),
        c=np.asarray(c, np.float32),
        Wxi=np.asarray(Wxi, np.float32), bxi=np.asarray(bxi, np.float32),
        Wxo=np.asarray(Wxo, np.float32), bxo=np.asarray(bxo, np.float32),
        Wxf=np.asarray(Wxf, np.float32), bxf=np.asarray(bxf, np.float32),
        Wxg=np.asarray(Wxg, np.float32), bxg=np.asarray(bxg, np.float32),
        Whi=np.asarray(Whi, np.float32), bhi=np.asarray(bhi, np.float32),
        Who=np.asarray(Who, np.float32), bho=np.asarray(bho, np.float32),
        Whf=np.asarray(Whf, np.float32), bhf=np.asarray(bhf, np.float32),
        Whg=np.asarray(Whg, np.float32), bhg=np.asarray(bhg, np.float32),
    )
    in_maps = make_in_maps(**args)
    nc = _get_nc(BL)
    res = run_bass_kernel_spmd(nc, in_maps, core_ids=list(range(NCORES)))
    h_new = np.empty((B_FULL, HID), np.float32)
    c_new = np.empty((B_FULL, HID), np.float32)
    for i in range(NCORES):
        s = slice(i * BL, (i + 1) * BL)
        h_new[s] = post_core(res.results[i]["h_out"])
        c_new[s] = post_core(res.results[i]["c_out"])
    return (h_new, c_new)


# revision 33
# speedup vs baseline: 1.1759x; 1.0115x over previous
"""LSTMCell (B=16384, IN=HID=512) on 8 TRN2 NeuronCores.

Strategy: data-parallel over batch (2048 rows/core), weights replicated.
Host pre-packs operands so the device kernel needs zero transposes:
  - GEMM computed as gates.T = W_cat.T @ [x;h].T  (K=1024 on partitions)
  - x/h/W/c cast to bf16 on host (fp32 PSUM accumulation on PE)
  - outputs round to bf16 on-chip (within the 2e-2 max-abs budget)

The PE floor for this GEMM is 512 MMs x 512 cols ~= 110.7us at bf16
(fp8 DoubleRow measured at the same 217ns/instr = only 2x FLOPs, and
the max-abs error budget needs 3 fp8 passes = 1.5x bf16 -> fp8 is out).
The matmul stream must run gap-free at that floor; everything else is
the head (engine prologue ~6.7us + first-chunk DMA) and the tail (last
eviction chain + output DMA + teardown). Design points, all measured
on traces:
  - every DMA region is CONTIGUOUS in DRAM (strided column slices and
    small standalone tensors both measured ~100-120GB/s vs ~250GB/s).
  - one hw-DGE queue sustains ~120-160GB/s (fabric-state dependent),
    and DMA triggers BLOCK the issuing engine's SEQ when the DGE queue
    is full, so the scalar queue (shared with the ACT engine) carries
    only the small early transfers, and the two ACT-table preloads go
    FIRST there (each lazy ACT_TABLE_LOAD is 1.3us; the table loads
    run on the ACT unit while the SEQ issues triggers concurrently).
  - sync queue carries ONLY the sixteen 256KB weight chunks in
    consumption order (splitting them into 128KB halves halved the
    effective queue rate -- per-trigger overhead), then chunks 2/3.
  - phase A: batch-chunk 0 k-outer. First r0+r1 as an r-pair across
    all 8 PSUM banks (8 MMs per weight chunk = 148GB/s demand, the
    queue's capacity; the final k-sweep runs rl-major so r0's banks
    stop ~0.9us early for their eviction chain). Then r2+r3 as a
    STAGGERED second r-pair: r2 alone k=0..4 (its 4 banks free just in
    time), r2+r3 interleaved k=5..7 (halves the rh=1 arrival demand
    exactly where the sync queue is still catching up), r3's k=0..4
    last on long-resident chunks (PSUM accumulation order is free:
    r3 starts its group at k=5 and stops at k=4).
  - ~34 warmup MMs (dep: one memset only) keep the PE activity monitor
    busy from prologue end (~7.1us) so the clock is ramped when the
    first data lands (~10.5-11.5us).
  - per-group gate order (f,i,g,o); the ACT queue is strictly in-order
    and tanh waits on the DVE adds, so o's eviction ACT is emitted
    BEFORE tanh -- banks free ~1.2us earlier for the next group.
    c_new is written bf16 directly by the DVE add (no cast op).
  - the LAST group runs f,g full-width, i and o as half-col bursts
    (second halves in the spare PSUM half's banks) so the i->c->tanh
    chains finish during the o bursts and only ACT-o(256)+mul+trigger
    trail the final MM; its output DMAs queue back-to-back on sync so
    the DGE pipelines their ~1.8us init latencies.
Result: exec ~128-131.5us depending on DMA-fabric state (baseline
129.9-132.5 under the same states); stream gap-free outside the
delivery-bound phase-A window, tail ~5.2us (was 6.3).
"""

import sys

sys.path.insert(0, "/opt/trn_rl_repo")

from contextlib import ExitStack

import ml_dtypes
import numpy as np

import concourse.bass as bass  # noqa: F401  (bass types used via bacc/mybir)
import concourse.mybir as mybir
import concourse.tile as tile
from concourse import bacc
from concourse.bass_utils import run_bass_kernel_spmd

B_FULL, IN, HID = 16384, 512, 512
NCORES = 8
BL = B_FULL // NCORES  # 2048 batch rows per core
JW = 512               # batch columns per chunk (matmul free dim)
P = 128

BF16 = mybir.dt.bfloat16
F32 = mybir.dt.float32
AF = mybir.ActivationFunctionType
BF16_NP = ml_dtypes.bfloat16

NK = (IN + HID) // P   # 8  k-chunks of the contraction dim
NR = HID // P          # 4  row-blocks of H per gate
NM = 4 * HID // P      # 16 gate-row blocks total (i,g,f,o order)

# gate burst order per group: f first (longest elementwise suffix),
# o last (shortest suffix: just ACT + mul + dma)
GATE_ORDER = (2, 0, 1, 3)  # f, i, g, o   (gate index: 0=i 1=g 2=f 3=o)


def build_nc(bl=BL):
    """Build the single-core Bass program (SPMD-replicated across cores)."""
    nbn = bl // JW
    n01 = min(nbn, 2)      # early chunks, k-granular
    n23 = max(nbn - 2, 0)  # late chunks, whole-chunk transfers
    nc = bacc.Bacc("TRN2", target_bir_lowering=False, debug=False)

    # chunk 0 packed k-major: [k][p][jw], each k-slice a contiguous
    # 128KB region (phase A needs per-k completion deps)
    xh0_in = nc.dram_tensor("xh0_in", [NK, P, JW], BF16, kind="ExternalInput")
    xh_in = nc.dram_tensor("xh_in", [max(nbn - 1, 1), P, NK * JW], BF16,
                           kind="ExternalInput")
    # w: contiguous 256KB chunk per (k, rhalf): [k][rh][p][(g*2+rl)*128+j]
    wt_in = nc.dram_tensor("wt_in", [NK, 2, P, 8 * P], BF16, kind="ExternalInput")
    # bias replicated 32x along free dim: 2KB per-partition lines
    bias_in = nc.dram_tensor("bias_in", [P, 32 * NM], F32, kind="ExternalInput")
    # c halves contiguous, bf16: [nb][rh][p][rl*JW + jw]
    c_in = nc.dram_tensor("c_in", [nbn, 2, P, 2 * JW], BF16, kind="ExternalInput")
    # outputs bf16, one contiguous [P, JW] block per (nb, r) group
    h_out = nc.dram_tensor("h_out", [nbn, NR, P, JW], BF16, kind="ExternalOutput")
    c_out = nc.dram_tensor("c_out", [nbn, NR, P, JW], BF16, kind="ExternalOutput")

    with ExitStack() as ctx:
        tc = ctx.enter_context(tile.TileContext(nc))
        wpool = ctx.enter_context(tc.tile_pool(name="w", bufs=1))
        xpool = ctx.enter_context(tc.tile_pool(name="xh", bufs=1))
        cpool = ctx.enter_context(tc.tile_pool(name="cin", bufs=1))
        gpool = ctx.enter_context(tc.tile_pool(name="gates", bufs=3))
        opool = ctx.enter_context(tc.tile_pool(name="outs", bufs=3))
        pspool = ctx.enter_context(tc.tile_pool(name="ps", bufs=1, space="PSUM"))

        ps = [
            pspool.tile([P, JW], F32, tag=f"p{i}", name=f"p{i}") for i in range(8)
        ]

        # --- warmup MMs: clock ramp needs ~3us of continuous PE activity
        wu = wpool.tile([P, P], BF16, tag="wu", name="wu")
        nc.vector.memset(wu[:], 0.0)
        wua = wpool.tile([P, 2], F32, tag="wua", name="wua")
        for _ in range(34):
            nc.tensor.matmul(ps[7][:, :P], wu[:], wu[:], start=True, stop=True)

        # --- SBUF tiles
        x0k = [
            xpool.tile([P, JW], BF16, tag=f"x0k{k}", name=f"x0k{k}")
            for k in range(NK)
        ]
        xh_t = [None] + [
            xpool.tile([P, NK * JW], BF16, tag=f"xh{nb}", name=f"xh{nb}")
            for nb in range(1, nbn)
        ]
        c_t = [
            [
                cpool.tile([P, 2 * JW], BF16, tag=f"c{nb}_{rh}", name=f"c{nb}_{rh}")
                for rh in range(2)
            ]
            for nb in range(nbn)
        ]
        bias_t = wpool.tile([P, 32 * NM], F32, tag="bias", name="bias")

        def xh_sl(nb, k):
            if nb == 0:
                return x0k[k][:]
            return xh_t[nb][:, k * JW : (k + 1) * JW]

        wts = [
            [
                wpool.tile([P, 8 * P], BF16, tag=f"w{k}_{rh}", name=f"w{k}_{rh}")
                for rh in range(2)
            ]
            for k in range(NK)
        ]

        # --- ACT-table preloads FIRST on the scalar queue: they occupy
        # the ACT unit (2x 1.3us) while the SEQ issues DMA triggers.
        nc.scalar.activation(wua[:], wu[:, :2], AF.Sigmoid)
        nc.scalar.activation(wua[:], wu[:, :2], AF.Tanh)
        # sync queue: ONLY weights, in consumption order -- phase A eats
        # one 256KB chunk per 1.73us (148GB/s), the queue's whole
        # measured capacity. Chunks 2/3 + their c follow (first use at
        # S+55us).
        for k in range(NK):
            nc.sync.dma_start(wts[k][0][:], wt_in[k, 0])
        # first two rh=1 chunks ride the scalar queue (it drains its
        # early load ~3us before r2 needs them; sync is still ~2MB
        # deep in rh=0 at that point) -- rest of rh=1 on sync.
        for k in range(2, NK):
            nc.sync.dma_start(wts[k][1][:], wt_in[k, 1])
        for nb in range(2, nbn):
            nc.sync.dma_start(xh_t[nb][:], xh_in[nb - 1])
            nc.sync.dma_start(c_t[nb][0][:], c_in[nb, 0])
            nc.sync.dma_start(c_t[nb][1][:], c_in[nb, 1])
        # scalar queue: x0 k-slices (74GB/s cadence), bias + c0 for the
        # first evictions (~S+12.6 on), then chunk 1 + its c (needed
        # S+27.6 / S+28). Total ~3.25MB, clear by ~33us; at most 3
        # triggers ride the DGE after the eviction-critical point so
        # the SEQ never blocks an eviction ACT on DGE back-pressure.
        for k in range(NK):
            nc.scalar.dma_start(x0k[k][:], xh0_in[k])
        nc.scalar.dma_start(bias_t[:], bias_in[:])
        # w01/w11 BEFORE the c0 halves: c only feeds the DVE c-chain
        # (never gates PSUM-bank release), while w01 gates the r2
        # phase's first PE sweep directly (measured 3us stall when the
        # c halves preceded it on a slow fabric).
        nc.scalar.dma_start(wts[0][1][:], wt_in[0, 1])
        nc.scalar.dma_start(wts[1][1][:], wt_in[1, 1])
        nc.scalar.dma_start(c_t[0][0][:], c_in[0, 0])
        nc.scalar.dma_start(c_t[0][1][:], c_in[0, 1])
        if nbn > 1:
            nc.scalar.dma_start(xh_t[1][:], xh_in[0])
            nc.scalar.dma_start(c_t[1][0][:], c_in[1, 0])
            nc.scalar.dma_start(c_t[1][1][:], c_in[1, 1])

        def mm_burst(nb, r, gg, base):
            """8 accumulating matmuls (k-inner) for gate gg of (nb, r)."""
            rh, rl = r // 2, r % 2
            col = (gg * 2 + rl) * P
            for k in range(NK):
                nc.tensor.matmul(
                    ps[base + gg][:],
                    wts[k][rh][:, col : col + P],
                    xh_sl(nb, k),
                    start=(k == 0),
                    stop=(k == NK - 1),
                )

        def mm_rpair_kouter(nb, rh):
            """Phase A: k-outer over r-pair (2rh, 2rh+1) across all 8
            PSUM banks, consuming weight chunks in arrival order. Gate g
            of r-local rl -> bank rl*4+g. The final k-sweep runs rl-major
            so r0's four banks all stop ~0.9us earlier -- their eviction
            chain gates the r2 phase's first sweep."""
            for k in range(NK):
                order = (
                    [(gg, rl) for gg in GATE_ORDER for rl in range(2)]
                    if k < NK - 1
                    else [(gg, rl) for rl in range(2) for gg in GATE_ORDER]
                )
                for gg, rl in order:
                    col = (gg * 2 + rl) * P
                    nc.tensor.matmul(
                        ps[rl * 4 + gg][:],
                        wts[k][rh][:, col : col + P],
                        xh_sl(nb, k),
                        start=(k == 0),
                        stop=(k == NK - 1),
                    )

        def mm_group_kouter(nb, r, base):
            """k-outer group: consumes weight chunks in arrival order."""
            rh, rl = r // 2, r % 2
            for k in range(NK):
                for gg in GATE_ORDER:
                    col = (gg * 2 + rl) * P
                    nc.tensor.matmul(
                        ps[base + gg][:],
                        wts[k][rh][:, col : col + P],
                        xh_sl(nb, k),
                        start=(k == 0),
                        stop=(k == NK - 1),
                    )

        def act_gate(nb, r, gg, base, sl, dst):
            fn = AF.Tanh if gg == 1 else AF.Sigmoid
            nc.scalar.activation(
                dst[:, sl], ps[base + gg][:, sl], fn,
                bias=bias_t[:, gg * NR + r : gg * NR + r + 1],
            )

        def elementwise(nb, r, base):
            """Cell update for group (nb, r); gates in ps[base..base+3].
            ACT eviction order f,i,g,o matches the bank-demand order of
            the next group using these banks. The DVE add writes c_new
            as bf16 directly (DMA'd out as-is; tanh reads the bf16)."""
            sl = slice(0, JW)
            cti = c_t[nb][r // 2]
            csl = slice((r % 2) * JW, (r % 2 + 1) * JW)
            ft = gpool.tile([P, JW], F32, tag="f")
            it = gpool.tile([P, JW], F32, tag="i")
            gt = gpool.tile([P, JW], F32, tag="g")
            ot = gpool.tile([P, JW], F32, tag="o")
            t1 = gpool.tile([P, JW], F32, tag="t1")
            t2 = gpool.tile([P, JW], F32, tag="t2")
            tch = gpool.tile([P, JW], F32, tag="tch")
            cnb = opool.tile([P, JW], BF16, tag="cnb")
            hnb = opool.tile([P, JW], BF16, tag="hnb")
            act_gate(nb, r, 2, base, sl, ft)
            nc.vector.tensor_mul(t2[:, sl], ft[:, sl], cti[:, csl])
            act_gate(nb, r, 0, base, sl, it)
            act_gate(nb, r, 1, base, sl, gt)
            nc.vector.tensor_mul(t1[:, sl], it[:, sl], gt[:, sl])
            nc.vector.tensor_add(cnb[:, sl], t1[:, sl], t2[:, sl])
            # o's ACT goes BEFORE tanh: the ACT queue is strictly
            # in-order and tanh waits on the DVE adds, which would hold
            # o's PSUM bank ~1.2us longer than needed.
            act_gate(nb, r, 3, base, sl, ot)
            nc.scalar.activation(tch[:, sl], cnb[:, sl], AF.Tanh)
            nc.sync.dma_start(c_out[nb, r], cnb[:, sl])
            nc.vector.tensor_mul(hnb[:, sl], ot[:, sl], tch[:, sl])
            nc.sync.dma_start(h_out[nb, r], hnb[:, sl])

        def last_group(nb, r, base):
            """Final group: minimize the post-last-MM serial chain."""
            rh, rl = r // 2, r % 2
            ob = 4 - base  # other PSUM half's banks, free by now
            for gg in (2, 1):  # f, g full-width
                mm_burst(nb, r, gg, base)
            cti = c_t[nb][rh]
            ft = gpool.tile([P, JW], F32, tag="f")
            it = gpool.tile([P, JW], F32, tag="i")
            gt = gpool.tile([P, JW], F32, tag="g")
            t1 = gpool.tile([P, JW], F32, tag="t1")
            t2 = gpool.tile([P, JW], F32, tag="t2")
            tch = gpool.tile([P, JW], F32, tag="tch")
            cnb = opool.tile([P, JW], BF16, tag="cnb")
            hnb = opool.tile([P, JW], BF16, tag="hnb")
            ot = gpool.tile([P, JW], F32, tag="o")
            hw_ = JW // 2
            sls = [slice(s * hw_, (s + 1) * hw_) for s in range(2)]
            csls = [slice(rl * JW + s * hw_, rl * JW + (s + 1) * hw_)
                    for s in range(2)]
            colI = (0 * 2 + rl) * P
            colO = (3 * 2 + rl) * P
            ibanks = [ps[base + 0], ps[ob + 0]]
            for s in range(2):
                for k in range(NK):
                    nc.tensor.matmul(
                        ibanks[s][:, sls[s]],
                        wts[k][rh][:, colI : colI + P],
                        xh_sl(nb, k)[:, sls[s]],
                        start=(k == 0),
                        stop=(k == NK - 1),
                    )
            obanks = [ps[base + 3], ps[ob + 3]]
            for s in range(2):
                for k in range(NK):
                    nc.tensor.matmul(
                        obanks[s][:, sls[s]],
                        wts[k][rh][:, colO : colO + P],
                        xh_sl(nb, k)[:, sls[s]],
                        start=(k == 0),
                        stop=(k == NK - 1),
                    )
            for s in range(2):
                act_gate(nb, r, 2, base, sls[s], ft)
                nc.vector.tensor_mul(t2[:, sls[s]], ft[:, sls[s]], cti[:, csls[s]])
            for s in range(2):
                act_gate(nb, r, 1, base, sls[s], gt)
            for s in range(2):
                nc.scalar.activation(
                    it[:, sls[s]], ibanks[s][:, sls[s]], AF.Sigmoid,
                    bias=bias_t[:, 0 * NR + r : 0 * NR + r + 1],
                )
                nc.vector.tensor_mul(t1[:, sls[s]], it[:, sls[s]], gt[:, sls[s]])
                nc.vector.tensor_add(cnb[:, sls[s]], t1[:, sls[s]], t2[:, sls[s]])
                nc.sync.dma_start(c_out[nb, r][:, sls[s]], cnb[:, sls[s]])
            for s in range(2):
                nc.scalar.activation(tch[:, sls[s]], cnb[:, sls[s]], AF.Tanh)
                nc.scalar.activation(
                    ot[:, sls[s]], obanks[s][:, sls[s]], AF.Sigmoid,
                    bias=bias_t[:, 3 * NR + r : 3 * NR + r + 1],
                )
                nc.vector.tensor_mul(hnb[:, sls[s]], ot[:, sls[s]], tch[:, sls[s]])
                nc.sync.dma_start(h_out[nb, r][:, sls[s]], hnb[:, sls[s]])

        # --- phase A: batch-chunk 0 entirely k-outer, consuming weight
        # chunks in arrival order. First the r0+r1 pair across all 8
        # banks, then r2 (r0's banks, freed gate-by-gate in matching
        # order) and r3 (r1's banks, freed long before).
        def phase_a2():
            """r2+r3 staggered second r-pair: r2 k-outer alone for
            k=0..4 (r0's banks free just in time; chunks banked), then
            k=5..7 interleaved with r3 (halves the rh=1 arrival demand
            right where the sync queue is still catching up), then r3's
            k=0..4 on long-resident chunks. Accumulation order within a
            bank is arbitrary: r3 starts its group at k=5 and stops at
            k=4."""
            def sweep(r, base, k, start, stop):
                rl = r % 2
                for gg in GATE_ORDER:
                    col = (gg * 2 + rl) * P
                    nc.tensor.matmul(
                        ps[base + gg][:],
                        wts[k][1][:, col : col + P],
                        xh_sl(0, k),
                        start=start,
                        stop=stop,
                    )
            for k in range(5):
                sweep(2, 0, k, k == 0, False)
            for k in range(5, NK):
                sweep(2, 0, k, False, k == NK - 1)
                sweep(3, 4, k, k == 5, False)
            for k in range(5):
                sweep(3, 4, k, False, k == 4)

        mm_rpair_kouter(0, 0)
        elementwise(0, 0, base=0)
        if nbn == 1:
            mm_group_kouter(0, 2, base=0)
            elementwise(0, 1, base=4)
            elementwise(0, 2, base=0)
            last_group(0, 3, base=4)
        else:
            elementwise(0, 1, base=4)
            phase_a2()
            elementwise(0, 2, base=0)
            elementwise(0, 3, base=4)

            # --- steady state: remaining groups, g-outer k-inner,
            # alternating PSUM halves for double buffering.
            groups = [
                (nb, r) for nb in range(1, nbn) for r in range(NR)
            ]
            for j, (nb, r) in enumerate(groups):
                base = 4 * (j % 2)
                if j == len(groups) - 1:
                    last_group(nb, r, base)
                else:
                    for gg in GATE_ORDER:
                        mm_burst(nb, r, gg, base)
                    elementwise(nb, r, base)

    nc.compile()
    return nc


def prep_shared(Wxi, Wxg, Wxf, Wxo, Whi, Whg, Whf, Who, bias_sum):
    """wt_in [NK,2,P,8P] bf16 and bias_in [P,32*NM] f32 (gate order i,g,f,o)."""
    Wx = np.concatenate([Wxi, Wxg, Wxf, Wxo], axis=0)  # [4H, IN]
    Wh = np.concatenate([Whi, Whg, Whf, Who], axis=0)  # [4H, HID]
    WT = np.concatenate([Wx.T, Wh.T], axis=0)          # [K=1024, 4H]
    W6 = WT.reshape(NK, P, 4, 2, 2, P)        # [k, p, g, rh, rl, j]
    wt_arr = np.ascontiguousarray(
        W6.transpose(0, 3, 1, 2, 4, 5)        # [k, rh, p, g, rl, j]
        .reshape(NK, 2, P, 8 * P)
        .astype(BF16_NP)
    )
    bias_arr = np.ascontiguousarray(
        np.tile(bias_sum.reshape(NM, P).T.astype(np.float32), (1, 32))
    )
    return wt_arr, bias_arr


def prep_core(x_s, h_s, c_s):
    """Per-core xh [nbn,P,NK*JW] bf16 and c [nbn,2,P,2*JW] bf16."""
    bl = x_s.shape[0]
    nbn = bl // JW
    xhT = np.concatenate([x_s, h_s], axis=1).T  # [K=1024, bl]
    xh_arr = np.ascontiguousarray(
        xhT.reshape(NK, P, nbn, JW).transpose(2, 1, 0, 3)
        .reshape(nbn, P, NK * JW)
        .astype(BF16_NP)
    )
    cT = c_s.T  # [HID, bl]
    c_arr = np.ascontiguousarray(
        cT.reshape(2, 2, P, nbn, JW).transpose(3, 0, 2, 1, 4)
        .reshape(nbn, 2, P, 2 * JW)
        .astype(BF16_NP)
    )
    return xh_arr, c_arr


def split_xh(xh_arr):
    """xh [nbn,P,NK*JW] -> (xh0 [NK,P,JW], xh_rest [nbn-1,P,NK*JW])."""
    nbn = xh_arr.shape[0]
    xh0 = np.ascontiguousarray(
        xh_arr[0].reshape(P, NK, JW).transpose(1, 0, 2)
    )
    if nbn > 1:
        rest = np.ascontiguousarray(xh_arr[1:])
    else:
        rest = np.zeros((1, P, NK * JW), BF16_NP)
    return xh0, rest


def post_core(arr):
    """[nbn,NR,P,JW] -> [bl, HID]"""
    arr = np.asarray(arr)
    nbn = arr.size // (NR * P * JW)
    arr = arr.reshape(nbn, NR, P, JW)
    return arr.transpose(0, 3, 1, 2).reshape(nbn * JW, HID)


_NC_CACHE = {}


def _get_nc(bl=BL):
    if bl not in _NC_CACHE:
        _NC_CACHE[bl] = build_nc(bl)
    return _NC_CACHE[bl]


def make_in_maps(x, h, c, Wxi, bxi, Wxo, bxo, Wxf, bxf, Wxg, bxg,
                 Whi, bhi, Who, bho, Whf, bhf, Whg, bhg, ncores=NCORES):
    bias_sum = np.concatenate(
        [bxi + bhi, bxg + bhg, bxf + bhf, bxo + bho], axis=0
    ).astype(np.float32)
    wt_arr, bias_arr = prep_shared(Wxi, Wxg, Wxf, Wxo, Whi, Whg, Whf, Who, bias_sum)
    bl = x.shape[0] // ncores
    in_maps = []
    for i in range(ncores):
        s = slice(i * bl, (i + 1) * bl)
        xh_arr, c_arr = prep_core(
            np.asarray(x[s], np.float32),
            np.asarray(h[s], np.float32),
            np.asarray(c[s], np.float32),
        )
        xh0, rest = split_xh(xh_arr)
        in_maps.append(
            {
                "xh0_in": xh0,
                "xh_in": rest,
                "wt_in": wt_arr,
                "bias_in": bias_arr,
                "c_in": c_arr,
            }
        )
    return in_maps


def kernel(x, h, c, Wxi, bxi, Wxo, bxo, Wxf, bxf, Wxg, bxg,
           Whi, bhi, Who, bho, Whf, bhf, Whg, bhg):
    args = dict(
        x=np.asarray(x, np.float32), h=np.asarray(h, np.float32),
        c=np.asarray(c, np.float32),
        Wxi=np.asarray(Wxi, np.float32), bxi=np.asarray(bxi, np.float32),
        Wxo=np.asarray(Wxo, np.float32), bxo=np.asarray(bxo, np.float32),
        Wxf=np.asarray(Wxf, np.float32), bxf=np.asarray(bxf, np.float32),
        Wxg=np.asarray(Wxg, np.float32), bxg=np.asarray(bxg, np.float32),
        Whi=np.asarray(Whi, np.float32), bhi=np.asarray(bhi, np.float32),
        Who=np.asarray(Who, np.float32), bho=np.asarray(bho, np.float32),
        Whf=np.asarray(Whf, np.float32), bhf=np.asarray(bhf, np.float32),
        Whg=np.asarray(Whg, np.float32), bhg=np.asarray(bhg, np.float32),
    )
    in_maps = make_in_maps(**args)
    nc = _get_nc(BL)
    res = run_bass_kernel_spmd(nc, in_maps, core_ids=list(range(NCORES)))
    h_new = np.empty((B_FULL, HID), np.float32)
    c_new = np.empty((B_FULL, HID), np.float32)
    for i in range(NCORES):
        s = slice(i * BL, (i + 1) * BL)
        h_new[s] = post_core(res.results[i]["h_out"])
        c_new[s] = post_core(res.results[i]["c_out"])
    return (h_new, c_new)
